# revision 30
# baseline (speedup 1.0000x reference)
"""Causal self-attention (GQA + RoPE) Trainium2 Bass kernel, 8-core SPMD.

Problem shapes (hardcoded): B=2, T=2048, C=2048, NH=16, NKV=4, HD=128.

Sharding: 8 cores = (batch b in {0,1}) x (kv-group g in {0..3}).
Core c = b*4+g handles batch b, q-heads 4g..4g+3, kv-head g.
  - Wq column-parallel (512 cols/core), Wk/Wv column-parallel (128 cols/core),
    Wproj row-parallel (512 rows/core) -> per-core partial [T, C] outputs,
    host sums the 4 partials per batch.

Per-core dataflow ("transposed flash"):
  - Inputs pre-transposed on host: xT [C, T].
  - Projections computed directly in transposed layout: qT/kT [HD, T]
    (lhsT = W chunk, rhs = xT chunk), V transposed to [T, HD] via PE.
  - RoPE on qT/kT via partition-shifted SBUF copies + cos/sin tables.
  - Scores computed transposed: S^T[tk, tq] = matmul(lhsT=kT block, rhs=qT),
    so softmax probs P^T are already in the layout the PV matmul needs as
    rhs -> no P transposes at all.
  - exp without max subtraction (scores are O(5) here; safe in fp32);
    row sums L[tq] via ones-vector matmul accumulated in PSUM.
  - The attention j-loop is software-pipelined two blocks ahead: S(j+2) is
    issued before PV(j)/L(j), so the Scalar-engine exp of a block has ~2 PE
    block-times to complete and the PE never waits on exp.
  - 1/L: reciprocal on the [1, W] row, then an async DMA DRAM round-trip
    broadcast to [P, W]; the normalization multiply is deferred by one head
    so the broadcast latency is fully hidden off the Vector queue.
  - Wproj consumes yT [HD, T] directly as lhsT. yT aliases qT storage.
"""

import numpy as np

import concourse.bass as bass
import concourse.bacc as bacc
import concourse.mybir as mybir
import concourse.tile as tile

B, T, C = 2, 2048, 2048
NH, NKV, HD = 16, 4, 128
P = 128
W = 512            # wide tile (PSUM bank = 512 fp32)
TB = T // P        # 16 t blocks
CB = C // P        # 16 c chunks
G = T // W         # 4 tq groups
NQ = 4             # q heads per core

F32 = mybir.dt.float32

USE_F32R = False          # kept for test.py compat; unused
MM = mybir.dt.bfloat16    # matmul-input compute dtype


def build_nc():
    nc = bacc.Bacc("TRN2", target_bir_lowering=False)
    xT = nc.dram_tensor("xT", (C, T), MM, kind="ExternalInput")[:]
    wq = nc.dram_tensor("wq", (C, NQ * HD), MM, kind="ExternalInput")[:]
    wk = nc.dram_tensor("wk", (C, HD), MM, kind="ExternalInput")[:]
    wv = nc.dram_tensor("wv", (C, HD), MM, kind="ExternalInput")[:]
    wp = nc.dram_tensor("wp", (NQ * HD, C), MM, kind="ExternalInput")[:]
    cosT = nc.dram_tensor("cosT", (P, T), F32, kind="ExternalInput")[:]
    msinT = nc.dram_tensor("msinT", (P, T), F32, kind="ExternalInput")[:]
    mask = nc.dram_tensor("mask", (P, W), F32, kind="ExternalInput")[:]
    ident = nc.dram_tensor("ident", (P, P), MM, kind="ExternalInput")[:]
    onescol = nc.dram_tensor("onescol", (P, 1), MM, kind="ExternalInput")[:]
    out = nc.dram_tensor("out", (T, C), MM, kind="ExternalOutput")[:]

    EXP = mybir.ActivationFunctionType.Exp

    with tile.TileContext(nc) as tc:
        with (
            tc.tile_pool(name="singles", bufs=1) as singles,
            tc.tile_pool(name="xin", bufs=8) as xin,
            tc.tile_pool(name="stage", bufs=3) as stage,
            tc.tile_pool(name="vst", bufs=2) as vstage,
            tc.tile_pool(name="ptp", bufs=6) as ptp,
            tc.tile_pool(name="outp", bufs=3) as outp,
            tc.tile_pool(name="small", bufs=4) as small,
            tc.tile_pool(name="dramp", bufs=3, space="DRAM") as dramp,
        ):
            # ---- resident tiles ----
            qT = singles.tile([P, NQ, T], MM)       # roped q; later reused as yT
            kT = singles.tile([P, T], MM)           # roped k, [hd, t]
            Vt = singles.tile([P, TB, HD], MM)      # [t_in_blk, blk, hd]
            cos_s = singles.tile([P, T], F32)
            msin_s = singles.tile([P, T], F32)
            mask_s = singles.tile([P, W], F32)      # [:, :128] tri, rest 0
            id_s = singles.tile([P, P], MM)
            ones_s = singles.tile([P, 1], MM)       # column of ones (lhsT)
            # grouped weight tiles: chunk 0 alone so phase A starts the
            # moment it lands; the rest in two large low-issue-count groups
            wg_sizes = [(0, 1), (1, 8), (8, 16)]
            wgrp = [singles.tile([P, hi - lo, 6 * P], MM, name=f"wgrp{lo}")
                    for lo, hi in wg_sizes]
            wall = []
            for c in range(CB):
                gi = 0 if c < 1 else (1 if c < 8 else 2)
                wall.append(wgrp[gi][:, c - wg_sizes[gi][0], :])
            wpall = singles.tile([P, NQ, C], MM)     # resident Wproj rows
            yT = qT                                  # alias: yT[:,h,t] overwrites

            # Spread the weight-preamble DMAs across engine queues so they
            # transfer in parallel and phase A's x loads own the Sync queue.
            wq3 = wq.rearrange("(cb p) m -> p cb m", p=P)
            wk3 = wk.rearrange("(cb p) m -> p cb m", p=P)
            wv3 = wv.rearrange("(cb p) m -> p cb m", p=P)
            for gq, (lo, hi) in enumerate(wg_sizes):
                eng = nc.scalar if gq != 1 else nc.sync
                csl4 = slice(lo, hi)
                eng.dma_start(out=wgrp[gq][:, :, 0:NQ * P],
                              in_=wq3[:, csl4, :])
                eng.dma_start(out=wgrp[gq][:, :, 4 * P:5 * P],
                              in_=wk3[:, csl4, :])
                eng.dma_start(out=wgrp[gq][:, :, 5 * P:6 * P],
                              in_=wv3[:, csl4, :])
            nc.scalar.dma_start(out=mask_s, in_=mask)
            nc.scalar.dma_start(out=id_s, in_=ident)
            nc.scalar.dma_start(out=ones_s, in_=onescol)
            nc.scalar.dma_start(out=cos_s, in_=cosT)
            nc.scalar.dma_start(out=msin_s, in_=msinT)
            wp3 = wp.rearrange("(hb p) c -> p hb c", p=P)
            for hb in range(NQ):
                nc.scalar.dma_start(out=wpall[:, hb, :], in_=wp3[:, hb, :])

            def rope_apply(dst, praw, tsl):
                # dst[d,:] = praw[d,:]*cos[d,:] + rot(praw)[d,:]*msin[d,:]
                # rot swaps halves; the rotate-half sign is folded into msin.
                tmp = stage.tile([P, W], F32, tag="ropetmp")
                nc.scalar.dma_start(out=tmp[0:64, :], in_=praw[64:128, :])
                nc.scalar.dma_start(out=tmp[64:128, :], in_=praw[0:64, :])
                nc.vector.tensor_mul(out=dst, in0=praw, in1=cos_s[:, tsl])
                nc.vector.tensor_mul(out=tmp, in0=tmp, in1=msin_s[:, tsl])
                nc.vector.tensor_add(out=dst, in0=dst, in1=tmp)

            # ---- phase A: QKV projections + rope + V transpose ----
            with tc.tile_pool(name="pa", bufs=1, space="PSUM") as pa:
                for t in range(G):
                    tsl = slice(t * W, (t + 1) * W)
                    ps = [pa.tile([P, W], F32, tag=f"a{m}", name=f"ps{m}")
                          for m in range(6)]
                    for c in range(CB):
                        x_c = xin.tile([P, W], MM, tag="x", name="x_c")
                        nc.sync.dma_start(out=x_c,
                                          in_=xT[c * P:(c + 1) * P, tsl])
                        st, sp = (c == 0), (c == CB - 1)
                        # m-order matches copy-out order (V, K, Q0..3) so the
                        # first matmul of t+1 waits only on the V copy.
                        for m in (5, 4, 0, 1, 2, 3):
                            nc.tensor.matmul(ps[m],
                                             wall[c][:, m * P:(m + 1) * P],
                                             x_c, start=st, stop=sp)
                    # All 6 PSUM-freeing copies first (alternating engines)
                    # so the next t-group's / phase B's matmuls are never
                    # WAR-blocked on rope work; ropes and transposes follow.
                    vraw = vstage.tile([P, W], MM, tag="vraw", name="vraw")
                    nc.vector.tensor_copy(out=vraw, in_=ps[5])
                    kraw = stage.tile([P, W], F32, tag="raw", bufs=6,
                                      name="kraw")
                    nc.scalar.copy(out=kraw, in_=ps[4])
                    qraws = []
                    for m in range(NQ):
                        qraw = stage.tile([P, W], F32, tag="raw", bufs=6,
                                          name="qraw")
                        if m % 2 == 0:
                            nc.vector.tensor_copy(out=qraw, in_=ps[m])
                        else:
                            nc.scalar.copy(out=qraw, in_=ps[m])
                        qraws.append(qraw)
                    for jj in range(4):
                        j = t * 4 + jj
                        pvt = pa.tile([P, W], MM, tag="pvt", bufs=2,
                                      name="pvt")
                        nc.tensor.transpose(pvt[:, :P],
                                            vraw[:, jj * P:(jj + 1) * P],
                                            id_s)
                        nc.vector.tensor_copy(out=Vt[:, j, :], in_=pvt[:, :P])
                    rope_apply(kT[:, tsl], kraw, tsl)
                    for m in range(NQ):
                        rope_apply(qT[:, m, tsl], qraws[m], tsl)

            # ---- phases B+C psum pools (A's pool released above) ----
            bc = ctx_bc = __import__("contextlib").ExitStack()
            pmm = ctx_bc.enter_context(
                tc.tile_pool(name="pmm", bufs=3, space="PSUM"))
            pacc = ctx_bc.enter_context(
                tc.tile_pool(name="pacc", bufs=3, space="PSUM"))
            plps = ctx_bc.enter_context(
                tc.tile_pool(name="plps", bufs=2, space="PSUM"))

            # ---- phase B: attention, transposed-flash, sw-pipelined ----
            # Normalization of head h is split in two deferred stages:
            # recip+broadcast issue right at h's end (GpSimd runs them in the
            # background); the Vector multiply issues a full head later, when
            # rb is long since ready, so it never head-blocks the Vector
            # queue in front of the next head's mask adds.
            def norm_stage1(pend):
                # tiny [1,W] reciprocal, then broadcast across partitions via
                # an async DRAM round-trip (partition-stride-0 read); latency
                # is hidden by the one-head deferral of stage2.
                pyps, plps_t, pgsl, ph = pend
                lsb = small.tile([1, W], F32, tag="lsb", name="lsb")
                nc.vector.reciprocal(out=lsb, in_=plps_t)
                rd = dramp.tile([1, W], F32, tag="rd", name="rd")
                nc.sync.dma_start(out=rd, in_=lsb)
                rb = stage.tile([P, W], F32, tag="rb", name="rb")
                nc.sync.dma_start(
                    out=rb,
                    in_=bass.AP(tensor=rd.tensor, offset=rd.offset,
                                ap=[[0, P]] + [list(dd) for dd in rd.ap[1:]]))
                return (pyps, rb, pgsl, ph)

            def norm_stage2(pend):
                pyps, rb, pgsl, ph = pend
                nc.vector.tensor_mul(out=yT[:, ph, pgsl], in0=pyps, in1=rb)

            pend1 = pend2 = None
            for g in range(G):
                gsl = slice(g * W, (g + 1) * W)
                nblk = 4 * (g + 1)   # causal: tk blocks 0..4(g+1)-1
                for h in range(NQ):
                    yps = pacc.tile([P, W], F32, tag="acc", name="yps")
                    lps = plps.tile([1, W], F32, tag="lps", name="lps")

                    def s_exp(j):
                        # S^T matmul for block j, then exp -> pt (bf16).
                        # Diagonal blocks: preload the causal mask into PSUM
                        # (Scalar) and accumulate S on top (start=False), so
                        # the busy Vector queue never gates the exp chain.
                        jj = j - g * 4   # >=0: j is in the diagonal chunk
                        vs = max(jj, 0) * P
                        sps = pmm.tile([P, W], F32, tag="mm", name="sps")
                        if jj >= 0:
                            nc.scalar.copy(out=sps[:, vs:W],
                                           in_=mask_s[:, 0:W - vs])
                        nc.tensor.matmul(sps[:, vs:W],
                                         kT[:, j * P:(j + 1) * P],
                                         qT[:, h, g * W + vs:(g + 1) * W],
                                         start=(jj < 0), stop=True)
                        pt = ptp.tile([P, W], MM, tag="pt", name="pt")
                        if jj < 0:
                            nc.scalar.activation(out=pt, in_=sps, func=EXP)
                        else:
                            nc.scalar.activation(out=pt[:, vs:W],
                                                 in_=sps[:, vs:W], func=EXP)
                        return pt, vs

                    def pv_l(pt, vs, j):
                        nc.tensor.matmul(yps[:, vs:W], Vt[:, j, :],
                                         pt[:, vs:W],
                                         start=(j == 0), stop=(j == nblk - 1))
                        nc.tensor.matmul(lps[:, vs:W], ones_s, pt[:, vs:W],
                                         start=(j == 0), stop=(j == nblk - 1))

                    # depth-2 pipeline: S runs two blocks ahead of PV/L so
                    # each exp has ~2 PE block-times to complete
                    from collections import deque
                    q = deque()
                    for j in range(nblk):
                        cur = s_exp(j)
                        q.append((cur[0], cur[1], j))
                        if len(q) > 2:
                            pv_l(*q.popleft())
                    while q:
                        pv_l(*q.popleft())

                    # stage1 first: its reciprocal never stalls, so the
                    # Vector queue flows into stage2's multiply, whose
                    # broadcast was launched a full head earlier.
                    nxt2 = norm_stage1(pend1) if pend1 is not None else None
                    if pend2 is not None:
                        norm_stage2(pend2)
                    pend2 = nxt2
                    pend1 = (yps, lps, gsl, h)
            if pend2 is not None:
                norm_stage2(pend2)
            norm_stage2(norm_stage1(pend1))

            # ---- phase C: output projection (row-parallel partial) ----
            for cc in range(4):
                csl = slice(cc * W, (cc + 1) * W)
                for i in range(TB):
                    # reuse pmm's banks: B's sps tiles free fast, while B's
                    # yps tiles are pinned by the deferred normalization
                    ops = pmm.tile([P, W], F32, tag="mm", name="ops")
                    for hb in range(NQ):
                        nc.tensor.matmul(ops,
                                         yT[:, hb, i * P:(i + 1) * P],
                                         wpall[:, hb, csl],
                                         start=(hb == 0), stop=(hb == NQ - 1))
                    ost = outp.tile([P, W], MM, tag="ost", name="ost")
                    nc.scalar.copy(out=ost, in_=ops)
                    deng = nc.sync if i % 2 == 0 else nc.scalar
                    deng.dma_start(out=out[i * P:(i + 1) * P, csl], in_=ost)
            ctx_bc.close()

    nc.compile()
    return nc


def make_tables():
    inv = (10000.0 ** (-(np.arange(64, dtype=np.float32) / np.float32(64.0)))
           ).astype(np.float32)
    freqs = np.arange(T, dtype=np.float32)[:, None] * inv[None, :]   # [T, 64]
    cos64 = np.cos(freqs).T.astype(np.float32)                       # [64, T]
    sin64 = np.sin(freqs).T.astype(np.float32)
    cosT = np.concatenate([cos64, cos64], axis=0)                    # [128, T]
    msinT = np.concatenate([-sin64, sin64], axis=0)
    mask = np.zeros((P, W), dtype=np.float32)
    mask[:, :P] = np.where(
        np.arange(P)[:, None] <= np.arange(P)[None, :],
        np.float32(0.0), np.float32(-1e5)).astype(np.float32)
    ident = np.eye(P, dtype=np.float32)
    return cosT, msinT, mask, ident


def shard_inputs(x, Wq, Wk, Wv, Wproj):
    import ml_dtypes
    bf16 = ml_dtypes.bfloat16
    cosT, msinT, mask, ident = make_tables()
    scale = np.float32(1.0 / np.sqrt(np.float32(HD)))
    xTb = [np.ascontiguousarray(x[b].T).astype(bf16) for b in range(B)]
    in_maps = []
    for core in range(8):
        b, g = core // 4, core % 4
        in_maps.append({
            "xT": xTb[b],
            "wq": np.ascontiguousarray(
                Wq[:, g * NQ * HD:(g + 1) * NQ * HD] * scale).astype(bf16),
            "wk": np.ascontiguousarray(Wk[:, g * HD:(g + 1) * HD]).astype(bf16),
            "wv": np.ascontiguousarray(Wv[:, g * HD:(g + 1) * HD]).astype(bf16),
            "wp": np.ascontiguousarray(
                Wproj[g * NQ * HD:(g + 1) * NQ * HD, :]).astype(bf16),
            "cosT": cosT, "msinT": msinT, "mask": mask,
            "ident": ident.astype(bf16),
            "onescol": np.ones((P, 1), dtype=bf16),
        })
    return in_maps


_NC_CACHE = {}


def _get_nc():
    key = USE_F32R
    if key not in _NC_CACHE:
        _NC_CACHE[key] = build_nc()
    return _NC_CACHE[key]


def kernel(x, Wq, Wk, Wv, Wproj):
    from concourse.bass_utils import run_bass_kernel_spmd
    x = np.asarray(x, dtype=np.float32)
    Wq = np.asarray(Wq, dtype=np.float32)
    Wk = np.asarray(Wk, dtype=np.float32)
    Wv = np.asarray(Wv, dtype=np.float32)
    Wproj = np.asarray(Wproj, dtype=np.float32)
    nc = _get_nc()
    in_maps = shard_inputs(x, Wq, Wk, Wv, Wproj)
    res = run_bass_kernel_spmd(nc, in_maps, core_ids=list(range(8)))
    out = np.zeros((B, T, C), dtype=np.float32)
    for core in range(8):
        b = core // 4
        out[b] += np.asarray(res.results[core]["out"], dtype=np.float32)
    return out
